# revision 36
# baseline (speedup 1.0000x reference)
"""Fused pre-norm attention kernel for Trainium2, sharded over 8 NeuronCores.

Problem: out = (LayerNorm(x) @ w_qkv -> multi-head attention) @ w_out
  x [4, 2048, 512], 8 heads x 64 dim, fp32.

Sharding: core c computes batch b = c//2 with head group g = c%2 (4 heads).
Each core produces a partial output [2048, 512] (its heads' contribution to
the out-projection); the host sums the two partials per batch.

Per-core kernel (~253us HW, rel err ~8.6e-3):
  1. LayerNorm token-major (bn_stats). rstd = exp(-0.5*ln(var+eps)) --
     NOT Sqrt: the ACT Sqrt table's 65536-ULP budget measurably doubles
     the kernel's final error. Ln/Exp live in different ACT table sets
     and the Tile scheduler interleaves single-tile ops (2.6us of table
     loads per tile!), so variances are DATA-batched: one Ln and one Exp
     instruction per group of 2-4 tiles (first batch small to unblock the
     pipeline head early). Normalized x cast to fp16, PE-transposed (4
     transposes into one psum tile), one wide ACT copy per token tile.
  2. q/k projections: head-pair 0 upfront, rest streamed into the attention
     loop's PE slack (deferred generator). All 16-bit matmul operands
     (fp16 pipeline; bf16 for the out-projection path).
  3. Attention in S^T layout, head-paired row-group packing: the two heads'
     K=64 S^T matmuls land on disjoint row groups and stream CONCURRENTLY.
     The exp over each [128, 2x512] score tile is SPLIT across engines:
     12/16 key blocks on ACT (exact exp), 4/16 on the Vector engine via
     the Schraudolph bit-trick -- i16 = round(S*A + B) bitcast as fp16 IS
     e^(S*SCALE) to +-3% -- one tensor_scalar op per tile. This keeps the
     per-key-block pipeline PE-paced instead of ACT-paced (ACT's exp
     stream runs at a fixed 1.2 GHz elem/cycle/lane).
     (A 4-way half-contraction P@V split that would also pack PV pairs
     onto row groups deadlocks NRT at runtime -- three orderings tried.)
  4. Softmax normalization (lazy, off the hot FIFOs): staging copies split
     ACT/DVE free the PSUM banks immediately; the denominator row is
     DMA-reshaped to [128, 4] so the reciprocal uses all DVE lanes (the
     old [1, 512] reciprocal burned 2us of 1-lane DVE per unit), then
     DMA-broadcast (partition-stride-0) back as bf16; 2x-rate bf16
     multiply. The last unit takes a direct path (no DMA reshape) so
     phase D's final tiles unblock sooner.
  5. Out-projection pinned on a covering attention flush + HAM warm-keepers.
"""

import os
import sys
from contextlib import ExitStack

import numpy as np

for _p in ("/opt/trn_rl_repo",):
    if _p not in sys.path and os.path.isdir(_p):
        sys.path.insert(0, _p)

import concourse.bacc as bacc
import concourse.bass as bass
import concourse.bass_utils as _bass_utils
import concourse.mybir as mybir
import concourse.tile as tile
from concourse.bass_utils import run_bass_kernel_spmd
from concourse.masks import make_identity

# Enable walrus's LDWEIGHTS optimization for this kernel's compilation:
# back-to-back matmuls with fresh stationary operands otherwise serialize
# on foreground weight loads (~100ns per matmul across ~750 matmuls).
if os.environ.get("BASS_LDW_OPT", "0") == "1" and not getattr(
    _bass_utils.run_command, "_ldw_patched", False
):
    _orig_run_command = _bass_utils.run_command

    def _run_command_ldw(cmd, cwd=None):
        cmd = [
            "--enable-ldw-opt=true" if c == "--enable-ldw-opt=false" else c
            for c in cmd
        ]
        return _orig_run_command(cmd, cwd=cwd)

    _run_command_ldw._ldw_patched = True
    _bass_utils.run_command = _run_command_ldw

F32 = mybir.dt.float32
F16 = mybir.dt.float16
BF16 = mybir.dt.bfloat16
I16 = mybir.dt.int16
AF = mybir.ActivationFunctionType

N_CORES = 8
B, N, D = 4, 2048, 512
H_PER_CORE = 4
DH = 64
GCOLS = H_PER_CORE * DH          # 256 columns per head-group
WCOLS = 3 * GCOLS                # 768 qkv columns per core
SCALE = DH ** -0.5
EPS = 1e-5
P = 128                          # SBUF partitions
NT = N // P                      # 16 token tiles
KT = D // P                      # 4 feature (contraction) tiles
QTW = 512                        # query-slice width for attention
NQT = N // QTW                   # 4 query slices

# Schraudolph fp16 exp: bitcast(int16(s*A + B)) ~ exp(s*SCALE), max rel err
# ~3.0%. Valid for raw scores in (-88.5, +88.9); data gives ~|74|.
A_SCHR = SCALE * 1024.0 / float(np.log(2.0))
B_SCHR = 15315.5

# which kb slots (0..15) of each attention unit run their exp on the DVE
# via Schraudolph instead of on ACT. Placed late in the unit so they never
# queue behind the previous unit's normalize work.
_DVE_KB_N = int(os.environ.get("BASS_DVE_KB", "4"))
DVE_KBS = set(13 - 2 * i for i in range(_DVE_KB_N))


def _build_nc():
    nc = bacc.Bacc(None)
    x_d = nc.declare_dram_parameter("x", [N, D], F32, isOutput=False)
    wqkv_d = nc.declare_dram_parameter("wqkv", [D, WCOLS], F16, isOutput=False)
    bqkv_d = nc.declare_dram_parameter("bqkv", [WCOLS, 1], F32, isOutput=False)
    wout_d = nc.declare_dram_parameter("wout", [GCOLS, D], BF16, isOutput=False)
    out_d = nc.declare_dram_parameter("out", [N, D], F32, isOutput=True)

    with tile.TileContext(nc, pool_alloc_mode="queue") as tc, ExitStack() as ctx:
        singles = ctx.enter_context(tc.tile_pool(name="singles", bufs=1))
        xin = ctx.enter_context(tc.tile_pool(name="xin", bufs=NT))
        xfp = ctx.enter_context(tc.tile_pool(name="xfp", bufs=3))
        stats = ctx.enter_context(tc.tile_pool(name="stats", bufs=4))
        pP = ctx.enter_context(tc.tile_pool(name="pP", bufs=6))
        smalls = ctx.enter_context(tc.tile_pool(name="smalls", bufs=8))
        outp = ctx.enter_context(tc.tile_pool(name="outp", bufs=3))
        psA = ctx.enter_context(tc.tile_pool(name="psA", bufs=4, space="PSUM"))
        psS = ctx.enter_context(tc.tile_pool(name="psS", bufs=2, space="PSUM"))
        dscr = ctx.enter_context(tc.tile_pool(name="dscr", bufs=6, space="DRAM"))

        ident = singles.tile([P, P], F16)
        make_identity(nc, ident)
        eps_sb = singles.tile([P, 1], F32)
        nc.vector.memset(eps_sb, EPS)

        # persistent SBUF tensors
        xT = singles.tile([P, KT, N], F16)              # xn^T  [feat, token]
        qkT = singles.tile([P, 4, N], F16)              # [qT(2 tiles), kT(2 tiles)]
        v_aug = singles.tile([P, NT, H_PER_CORE, DH + 1], F16)
        oT = singles.tile([P, 2, N], BF16)              # O^T rows (4 heads x 64)
        w_sb = singles.tile([P, KT, WCOLS], F16)
        bias_sb = singles.tile([P, 6], F32)
        oc_all = singles.tile([DH + 1, NQT * H_PER_CORE, QTW], BF16)
        vbias_sb = singles.tile([P, GCOLS], F32)
        wout_sb = singles.tile([P, 2, D], BF16)

        # x tiles stream in first (phase A is the pipeline head); weights
        # follow on the same queue, ordered by first use.
        x_tiles = []
        for tt in range(NT):
            x_tile = xin.tile([P, D], F32, tag="xt")
            nc.sync.dma_start(out=x_tile, in_=x_d[tt * P : (tt + 1) * P, :])
            x_tiles.append(x_tile)

        nc.sync.dma_start(out=w_sb, in_=wqkv_d[:, :].rearrange("(kt p) m -> p kt m", p=P))
        nc.sync.dma_start(out=bias_sb, in_=bqkv_d[:, :].rearrange("(t p) o -> p (t o)", p=P))
        bq = bqkv_d[:, :]
        vbias_bcast = bass.AP(
            tensor=bq.tensor, offset=2 * GCOLS, ap=[[0, P], [1, GCOLS]]
        )
        nc.sync.dma_start(out=vbias_sb, in_=vbias_bcast)
        nc.sync.dma_start(out=wout_sb, in_=wout_d[:, :].rearrange("(ki p) n -> p ki n", p=P))

        # ones columns of v_aug
        ones_sb = singles.tile([P, 1], F32)
        nc.vector.memset(ones_sb, 1.0)
        nc.vector.tensor_copy(
            out=v_aug[:, :, :, DH : DH + 1],
            in_=ones_sb.to_broadcast((P, NT, H_PER_CORE, 1)),
        )

        # PE matmuls accept only ONE sync wait command. Sacrificial ldweights
        # ops make the PE observe fresh semaphore ticks so real matmuls keep
        # to one wait.
        def pe_observe(ap):
            nc.tensor.ldweights(ap.bitcast(BF16))

        pe_observe(ident[:, 0:1])
        pe_observe(w_sb[:, 0, 0:1])
        pe_observe(wout_sb[:, 0, 0:1])

        # ---- Phase A: LayerNorm + transpose ----------------------------
        # rstd = exp(-0.5 * ln(var + eps)). All Ln ops are emitted before
        # all Exp ops so ACT loads each table set exactly once (and the
        # attention exps reuse the exp set). The Sqrt table is NOT used:
        # its 65536-ULP budget measurably doubles the kernel's error.
        mv_all = singles.tile([P, NT, 2], F32)
        lv_all = singles.tile([P, NT], F32)
        rstd_all = singles.tile([P, NT], F32)
        for tt in range(NT):
            st = stats.tile([P, nc.vector.BN_STATS_DIM], F32)
            nc.vector.bn_stats(out=st, in_=x_tiles[tt])
            nc.vector.bn_aggr(out=mv_all[:, tt, :], in_=st)
            # Ln/Exp are in different ACT table sets and the scheduler
            # freely interleaves single-tile ops (one 2.6us table round
            # trip per tile). Batch 4 tiles' variances into ONE Ln and
            # ONE Exp instruction instead: few loads, early first batch.
            if tt in (1, 3, 7, 11, 15):
                bs = slice({1: 0, 3: 2, 7: 4, 11: 8, 15: 12}[tt], tt + 1)
                nc.scalar.activation(
                    out=lv_all[:, bs], in_=mv_all[:, bs, 1],
                    func=AF.Ln, bias=eps_sb,
                )
                nc.scalar.activation(
                    out=rstd_all[:, bs], in_=lv_all[:, bs],
                    func=AF.Exp, scale=-0.5,
                )
        for tt in range(NT):
            x_tile = x_tiles[tt]
            xf = xfp.tile([P, D], F16, tag="xf")
            nc.vector.tensor_scalar(
                out=xf,
                in0=x_tile,
                scalar1=mv_all[:, tt, 0:1],
                scalar2=rstd_all[:, tt : tt + 1],
                op0=mybir.AluOpType.subtract,
                op1=mybir.AluOpType.mult,
            )
            # 4 transposes into one psum tile, one wide ACT copy out
            psT = psA.tile([P, KT, P], F16, tag="ps")
            for ft in range(KT):
                nc.tensor.transpose(psT[:, ft, :], xf[:, ft * P : (ft + 1) * P], ident)
            nc.scalar.copy(out=xT[:, :, tt * P : (tt + 1) * P], in_=psT)
            # v projection for this token tile (its xT slices just landed)
            if tt >= 2:
                pe_observe(v_aug[:, tt - 2, 0, 0:1])
            ps = psA.tile([P, GCOLS], F32)
            for kt in range(KT):
                nc.tensor.matmul(
                    ps,
                    xT[:, kt, tt * P : (tt + 1) * P],
                    w_sb[:, kt, 2 * GCOLS : 3 * GCOLS],
                    start=(kt == 0),
                    stop=(kt == KT - 1),
                )
            nc.vector.tensor_add(
                out=v_aug[:, tt, :, 0:DH],
                in0=ps.rearrange("p (h d) -> p h d", h=H_PER_CORE),
                in1=vbias_sb.rearrange("p (h d) -> p h d", h=H_PER_CORE),
            )

        # PE observes the final xT copy tick before QKV matmuls
        pe_observe(xT[:, KT - 1, N - 1 : N])

        # ---- Phase B: q/k projections for head-pair 0 ------------------
        def qk_group_steps(mi, nt):
            ps = psA.tile([P, QTW], F32, tag="ps")
            for kt in range(KT):
                yield lambda kt=kt, ps=ps: nc.tensor.matmul(
                    ps,
                    w_sb[:, kt, mi * P : (mi + 1) * P],
                    xT[:, kt, nt * QTW : (nt + 1) * QTW],
                    start=(kt == 0),
                    stop=(kt == KT - 1),
                )
            yield lambda ps=ps: nc.vector.tensor_scalar(
                out=qkT[:, mi, nt * QTW : (nt + 1) * QTW],
                in0=ps,
                scalar1=bias_sb[:, mi : mi + 1],
                scalar2=None,
                op0=mybir.AluOpType.add,
            )

        for mi in (0, 2, 1, 3):
            for nt in range(NQT):
                if (mi, nt) != (0, 0) and tuple(sorted((mi, nt))) != (0, 2):
                    pass
                for step in qk_group_steps(mi, nt):
                    step()

        pe_observe(qkT[:, 2, QTW - 1 : QTW])
        last_flush = {}
        mid_flush = {}

        # ---- Phase C: attention (S^T layout, head-paired) --------------
        for hp in range(2):
            h0, h1 = 2 * hp, 2 * hp + 1
            mi_q, mi_k = hp, 2 + hp
            for qt in range(NQT):
                qs = slice(qt * QTW, (qt + 1) * QTW)
                po0L = psA.tile([DH + 1, QTW], F32, tag="ps")
                po0H = psA.tile([DH + 1, QTW], F32, tag="ps")
                po1L = psA.tile([DH + 1, QTW], F32, tag="ps")
                po1H = psA.tile([DH + 1, QTW], F32, tag="ps")
                pending = None
                for kb in range(NT):
                    ks = slice(kb * P, (kb + 1) * P)
                    ps_s = psS.tile([P, 2, QTW], F32)
                    mm_s = nc.tensor.matmul(
                        ps_s[:, 0, :],
                        qkT[0:DH, mi_k, ks],
                        qkT[0:DH, mi_q, qs],
                        start=True,
                        stop=True,
                    )

                    nc.tensor.matmul(
                        ps_s[:, 1, :],
                        qkT[DH:P, mi_k, ks],
                        qkT[DH:P, mi_q, qs],
                        start=True,
                        stop=True,
                    )
                    pT = pP.tile([P, 2, QTW], F16)
                    if kb in DVE_KBS:
                        # Schraudolph: e^(s*SCALE) ~= bitcast_f16(i16(s*A + B))
                        nc.vector.tensor_scalar(
                            out=pT.bitcast(I16),
                            in0=ps_s,
                            scalar1=A_SCHR,
                            scalar2=B_SCHR,
                            op0=mybir.AluOpType.mult,
                            op1=mybir.AluOpType.add,
                        )
                    else:
                        nc.scalar.activation(out=pT, in_=ps_s, func=AF.Exp, scale=SCALE)
                    def pv(pkb, ppT, stop):
                        st = pkb == 0
                        nc.tensor.matmul(
                            po0L, v_aug[0:DH, pkb, h0, :], ppT[0:DH, 0, :],
                            start=st, stop=stop,
                        )
                        nc.tensor.matmul(
                            po1H, v_aug[DH:P, pkb, h1, :], ppT[DH:P, 1, :],
                            start=st, stop=stop,
                        )
                        nc.tensor.matmul(
                            po1L, v_aug[0:DH, pkb, h1, :], ppT[0:DH, 1, :],
                            start=st, stop=stop,
                        )
                        mm_last = nc.tensor.matmul(
                            po0H, v_aug[DH:P, pkb, h0, :], ppT[DH:P, 0, :],
                            start=st, stop=stop,
                        )
                        return mm_last, mm_last

                    if pending is not None:
                        pkb, ppT = pending
                        if pkb == 0:
                            pe_observe(ppT[:, 0, 0:1])
                        pv(pkb, ppT, stop=False)
                    pending = (kb, pT)
                pkb, ppT = pending
                _, last_att_mm = pv(pkb, ppT, stop=True)
                if hp == 1:
                    last_flush[qt] = last_att_mm
                # normalize both heads (lazy; only gates phase D). Both
                # staging copies go FIRST (frees the PSUM banks promptly);
                # the DMA-gated den chain then runs entirely on GpSimd so
                # the hot ACT/DVE FIFOs never block on a DMA dependency.
                u0, u1 = h0 * NQT + qt, h1 * NQT + qt
                nc.scalar.copy(out=oc_all[:, u0, :], in_=po0L)
                nc.vector.tensor_copy(out=oc_all[:, u1, :], in_=po1L)
                nc.vector.tensor_add(
                    out=oc_all[:, u0, :], in0=oc_all[:, u0, :], in1=po0H)
                nc.vector.tensor_add(
                    out=oc_all[:, u1, :], in0=oc_all[:, u1, :], in1=po1H)
                for h in (h0, h1):
                    u = h * NQT + qt
                    r0 = (h % 2) * DH
                    if hp == 1 and qt == NQT - 1:
                        # tail fast-path: direct 1-lane reciprocal (no DMA
                        # reshape latency) so phase D's last tiles unblock
                        # sooner after the final attention flush
                        rq = smalls.tile([1, QTW], BF16)
                        with nc.allow_low_precision(reason="softmax denom"):
                            nc.vector.reciprocal(out=rq, in_=oc_all[DH : DH + 1, u, :])
                        rd = dscr.tile([1, QTW], BF16)
                        nc.sync.dma_start(out=rd, in_=rq)
                        rb = smalls.tile([DH, QTW], BF16)
                        nc.sync.dma_start(out=rb, in_=rd.to_broadcast((DH, QTW)))
                        nc.vector.tensor_mul(
                            out=oT[r0 : r0 + DH, h // 2, qs],
                            in0=oc_all[0:DH, u, :],
                            in1=rb,
                        )
                        continue
                    dd = dscr.tile([1, QTW], BF16)
                    nc.sync.dma_start(out=dd, in_=oc_all[DH : DH + 1, u, :])
                    d128 = smalls.tile([P, QTW // P], BF16)
                    nc.sync.dma_start(
                        out=d128, in_=dd.rearrange("o (p j) -> (o p) j", p=P)
                    )
                    r128 = smalls.tile([P, QTW // P], BF16)
                    with nc.allow_low_precision(reason="softmax denom recip"):
                        nc.vector.reciprocal(out=r128, in_=d128)
                    rd = dscr.tile([1, QTW], BF16)
                    nc.sync.dma_start(
                        out=rd.rearrange("o (p j) -> (o p) j", p=P), in_=r128
                    )
                    rb = smalls.tile([DH, QTW], BF16)
                    nc.sync.dma_start(out=rb, in_=rd.to_broadcast((DH, QTW)))
                    nc.vector.tensor_mul(
                        out=oT[r0 : r0 + DH, h // 2, qs],
                        in0=oc_all[0:DH, u, :],
                        in1=rb,
                    )

        # Keep the PE HAM-warm across the normalize-chain tail
        for wk in range(12):
            ps = psA.tile([P, QTW], F32, tag="ps")
            nc.tensor.matmul(
                ps,
                qkT[0:DH, 0, 0:P],
                qkT[0:DH, 0, 0:QTW],
                start=True,
                stop=True,
            )

        pe_observe(oT[0:DH, 1, N - 1 : N].bitcast(F16))

        # ---- Phase D: out projection -----------------------------------
        ob_hist = []
        for tt in range(NT):
            if len(ob_hist) >= 2:
                pe_observe(ob_hist[-2][:, 0:1])
            ps = psA.tile([P, D], F32)
            for ki in range(2):
                mm = nc.tensor.matmul(
                    ps,
                    oT[:, ki, tt * P : (tt + 1) * P],
                    wout_sb[:, ki, :],
                    start=(ki == 0),
                    stop=(ki == 1),
                )
                pin = last_flush[min(tt // NQT + 1, NQT - 1)]
                tile.add_dep_helper(
                    mm.ins, pin.ins, sync=False,
                    reason="phase D after covering attention flush",
                )
            ob = outp.tile([P, D], F32)
            if tt % 2 == 0:
                nc.scalar.copy(out=ob, in_=ps)
            else:
                nc.vector.tensor_copy(out=ob, in_=ps)
            ob_hist.append(ob)
            nc.sync.dma_start(out=out_d[tt * P : (tt + 1) * P, :], in_=ob)

    nc.compile()
    return nc


_NC_CACHE = {}
last_results = None  # BassKernelResults of the most recent run (for test.py)


def _get_nc():
    key = (_DVE_KB_N,)
    if key not in _NC_CACHE:
        _NC_CACHE[key] = _build_nc()
    return _NC_CACHE[key]


def kernel(x, gamma, beta, w_qkv, w_out):
    global last_results
    import ml_dtypes

    x = np.ascontiguousarray(np.asarray(x, dtype=np.float32))
    gamma = np.asarray(gamma, dtype=np.float32)
    beta = np.asarray(beta, dtype=np.float32)
    w_qkv = np.asarray(w_qkv, dtype=np.float32)
    w_out = np.asarray(w_out, dtype=np.float32)

    # fold gamma/beta into the projection (exact algebra)
    wp = gamma[:, None] * w_qkv                      # [512, 1536]
    bp = beta @ w_qkv                                # [1536]

    in_maps = []
    for c in range(N_CORES):
        b = c // 2
        g = c % 2
        sl = [slice(s * D + g * GCOLS, s * D + (g + 1) * GCOLS) for s in range(3)]
        wg = np.concatenate([wp[:, s] for s in sl], axis=1)          # [512, 768]
        bg = np.concatenate([bp[s] for s in sl])[:, None]            # [768, 1]
        wo = w_out[g * GCOLS : (g + 1) * GCOLS, :]                   # [256, 512]
        in_maps.append(
            {
                "x": np.ascontiguousarray(x[b]),
                "wqkv": np.ascontiguousarray(wg.astype(np.float16)),
                "bqkv": np.ascontiguousarray(bg.astype(np.float32)),
                "wout": np.ascontiguousarray(wo.astype(ml_dtypes.bfloat16)),
            }
        )

    nc = _get_nc()
    last_results = run_bass_kernel_spmd(nc, in_maps, list(range(N_CORES)))
    outs = [m["out"] for m in last_results.results]
    out = np.stack([outs[2 * b] + outs[2 * b + 1] for b in range(B)])
    return np.ascontiguousarray(out.astype(np.float32))


# revision 38
# speedup vs baseline: 1.0624x; 1.0624x over previous
"""Fused pre-norm attention kernel for Trainium2, sharded over 8 NeuronCores.

Problem: out = (LayerNorm(x) @ w_qkv -> multi-head attention) @ w_out
  x [4, 2048, 512], 8 heads x 64 dim, fp32.

Sharding: core c computes batch b = c//2 with head group g = c%2 (4 heads).
Each core produces a partial output [2048, 512] (its heads' contribution to
the out-projection); the host sums the two partials per batch.

Per-core kernel (~253us HW, rel err ~8.6e-3):
  1. LayerNorm token-major (bn_stats). rstd = exp(-0.5*ln(var+eps)) --
     NOT Sqrt: the ACT Sqrt table's 65536-ULP budget measurably doubles
     the kernel's final error. Ln/Exp live in different ACT table sets
     and the Tile scheduler interleaves single-tile ops (2.6us of table
     loads per tile!), so variances are DATA-batched: one Ln and one Exp
     instruction per group of 2-4 tiles (first batch small to unblock the
     pipeline head early). Normalized x cast to fp16, PE-transposed (4
     transposes into one psum tile), one wide ACT copy per token tile.
  2. q/k projections: head-pair 0 upfront, rest streamed into the attention
     loop's PE slack (deferred generator). All 16-bit matmul operands
     (fp16 pipeline; bf16 for the out-projection path).
  3. Attention in S^T layout, head-paired row-group packing: the two heads'
     K=64 S^T matmuls land on disjoint row groups and stream CONCURRENTLY.
     The exp over each [128, 2x512] score tile is SPLIT across engines:
     12/16 key blocks on ACT (exact exp), 4/16 on the Vector engine via
     the Schraudolph bit-trick -- i16 = round(S*A + B) bitcast as fp16 IS
     e^(S*SCALE) to +-3% -- one tensor_scalar op per tile. This keeps the
     per-key-block pipeline PE-paced instead of ACT-paced (ACT's exp
     stream runs at a fixed 1.2 GHz elem/cycle/lane).
     (A 4-way half-contraction P@V split that would also pack PV pairs
     onto row groups deadlocks NRT at runtime -- three orderings tried.)
  4. Softmax normalization (lazy, off the hot FIFOs): staging copies split
     ACT/DVE free the PSUM banks immediately; the denominator row is
     DMA-reshaped to [128, 4] so the reciprocal uses all DVE lanes (the
     old [1, 512] reciprocal burned 2us of 1-lane DVE per unit), then
     DMA-broadcast (partition-stride-0) back as bf16; 2x-rate bf16
     multiply. The last unit takes a direct path (no DMA reshape) so
     phase D's final tiles unblock sooner.
  5. Out-projection pinned on a covering attention flush + HAM warm-keepers.
"""

import os
import sys
from contextlib import ExitStack

import numpy as np

for _p in ("/opt/trn_rl_repo",):
    if _p not in sys.path and os.path.isdir(_p):
        sys.path.insert(0, _p)

import concourse.bacc as bacc
import concourse.bass as bass
import concourse.bass_utils as _bass_utils
import concourse.mybir as mybir
import concourse.tile as tile
from concourse.bass_utils import run_bass_kernel_spmd
from concourse.masks import make_identity

# Enable walrus's LDWEIGHTS optimization for this kernel's compilation:
# back-to-back matmuls with fresh stationary operands otherwise serialize
# on foreground weight loads (~100ns per matmul across ~750 matmuls).
if os.environ.get("BASS_LDW_OPT", "0") == "1" and not getattr(
    _bass_utils.run_command, "_ldw_patched", False
):
    _orig_run_command = _bass_utils.run_command

    def _run_command_ldw(cmd, cwd=None):
        cmd = [
            "--enable-ldw-opt=true" if c == "--enable-ldw-opt=false" else c
            for c in cmd
        ]
        return _orig_run_command(cmd, cwd=cwd)

    _run_command_ldw._ldw_patched = True
    _bass_utils.run_command = _run_command_ldw

F32 = mybir.dt.float32
F16 = mybir.dt.float16
BF16 = mybir.dt.bfloat16
I16 = mybir.dt.int16
AF = mybir.ActivationFunctionType

N_CORES = 8
B, N, D = 4, 2048, 512
H_PER_CORE = 4
DH = 64
GCOLS = H_PER_CORE * DH          # 256 columns per head-group
WCOLS = 3 * GCOLS                # 768 qkv columns per core
SCALE = DH ** -0.5
EPS = 1e-5
P = 128                          # SBUF partitions
NT = N // P                      # 16 token tiles
KT = D // P                      # 4 feature (contraction) tiles
QTW = 512                        # query-slice width for attention
NQT = N // QTW                   # 4 query slices

# Schraudolph fp16 exp: bitcast(int16(s*A + B)) ~ exp(s*SCALE), max rel err
# ~3.0%. Valid for raw scores in (-88.5, +88.9); data gives ~|74|.
A_SCHR = SCALE * 1024.0 / float(np.log(2.0))
B_SCHR = 15315.5

# which kb slots (0..15) of each attention unit run their exp on the DVE
# via Schraudolph instead of on ACT. Placed late in the unit so they never
# queue behind the previous unit's normalize work.
_DVE_KB_N = int(os.environ.get("BASS_DVE_KB", "5"))
DVE_KBS = set(13 - 2 * i for i in range(_DVE_KB_N))


def _build_nc():
    nc = bacc.Bacc(None)
    x_d = nc.declare_dram_parameter("x", [N, D], F32, isOutput=False)
    wqkv_d = nc.declare_dram_parameter("wqkv", [D, WCOLS], F16, isOutput=False)
    bqkv_d = nc.declare_dram_parameter("bqkv", [WCOLS, 1], F32, isOutput=False)
    wout_d = nc.declare_dram_parameter("wout", [GCOLS, D], BF16, isOutput=False)
    out_d = nc.declare_dram_parameter("out", [N, D], F32, isOutput=True)

    with tile.TileContext(nc, pool_alloc_mode="queue") as tc, ExitStack() as ctx:
        singles = ctx.enter_context(tc.tile_pool(name="singles", bufs=1))
        xin = ctx.enter_context(tc.tile_pool(name="xin", bufs=NT))
        xfp = ctx.enter_context(tc.tile_pool(name="xfp", bufs=3))
        stats = ctx.enter_context(tc.tile_pool(name="stats", bufs=4))
        pP = ctx.enter_context(tc.tile_pool(name="pP", bufs=6))
        smalls = ctx.enter_context(tc.tile_pool(name="smalls", bufs=8))
        outp = ctx.enter_context(tc.tile_pool(name="outp", bufs=3))
        psA = ctx.enter_context(tc.tile_pool(name="psA", bufs=4, space="PSUM"))
        psS = ctx.enter_context(tc.tile_pool(name="psS", bufs=2, space="PSUM"))
        dscr = ctx.enter_context(tc.tile_pool(name="dscr", bufs=6, space="DRAM"))

        ident = singles.tile([P, P], F16)
        make_identity(nc, ident)
        eps_sb = singles.tile([P, 1], F32)
        nc.vector.memset(eps_sb, EPS)

        # persistent SBUF tensors
        xT = singles.tile([P, KT, N], F16)              # xn^T  [feat, token]
        qkT = singles.tile([P, 4, N], F16)              # [qT(2 tiles), kT(2 tiles)]
        v_aug = singles.tile([P, NT, H_PER_CORE, DH + 1], F16)
        oT = singles.tile([P, 2, N], BF16)              # O^T rows (4 heads x 64)
        w_sb = singles.tile([P, KT, WCOLS], F16)
        bias_sb = singles.tile([P, 6], F32)
        oc_all = singles.tile([DH + 1, NQT * H_PER_CORE, QTW], BF16)
        vbias_sb = singles.tile([P, GCOLS], F32)
        wout_sb = singles.tile([P, 2, D], BF16)

        # x tiles stream in first (phase A is the pipeline head); weights
        # follow on the same queue, ordered by first use.
        x_tiles = []
        for tt in range(NT):
            x_tile = xin.tile([P, D], F32, tag="xt")
            nc.sync.dma_start(out=x_tile, in_=x_d[tt * P : (tt + 1) * P, :])
            x_tiles.append(x_tile)

        nc.sync.dma_start(out=w_sb, in_=wqkv_d[:, :].rearrange("(kt p) m -> p kt m", p=P))
        nc.sync.dma_start(out=bias_sb, in_=bqkv_d[:, :].rearrange("(t p) o -> p (t o)", p=P))
        bq = bqkv_d[:, :]
        vbias_bcast = bass.AP(
            tensor=bq.tensor, offset=2 * GCOLS, ap=[[0, P], [1, GCOLS]]
        )
        nc.sync.dma_start(out=vbias_sb, in_=vbias_bcast)
        nc.sync.dma_start(out=wout_sb, in_=wout_d[:, :].rearrange("(ki p) n -> p ki n", p=P))

        # ones columns of v_aug
        ones_sb = singles.tile([P, 1], F32)
        nc.vector.memset(ones_sb, 1.0)
        nc.vector.tensor_copy(
            out=v_aug[:, :, :, DH : DH + 1],
            in_=ones_sb.to_broadcast((P, NT, H_PER_CORE, 1)),
        )

        # PE matmuls accept only ONE sync wait command. Sacrificial ldweights
        # ops make the PE observe fresh semaphore ticks so real matmuls keep
        # to one wait.
        def pe_observe(ap):
            nc.tensor.ldweights(ap.bitcast(BF16))

        pe_observe(ident[:, 0:1])
        pe_observe(w_sb[:, 0, 0:1])
        pe_observe(wout_sb[:, 0, 0:1])

        # ---- Phase A: LayerNorm + transpose ----------------------------
        # rstd = exp(-0.5 * ln(var + eps)). All Ln ops are emitted before
        # all Exp ops so ACT loads each table set exactly once (and the
        # attention exps reuse the exp set). The Sqrt table is NOT used:
        # its 65536-ULP budget measurably doubles the kernel's error.
        mv_all = singles.tile([P, NT, 2], F32)
        lv_all = singles.tile([P, NT], F32)
        rstd_all = singles.tile([P, NT], F32)
        for tt in range(NT):
            st = stats.tile([P, nc.vector.BN_STATS_DIM], F32)
            nc.vector.bn_stats(out=st, in_=x_tiles[tt])
            nc.vector.bn_aggr(out=mv_all[:, tt, :], in_=st)
            # Ln/Exp are in different ACT table sets and the scheduler
            # freely interleaves single-tile ops (one 2.6us table round
            # trip per tile). Batch 4 tiles' variances into ONE Ln and
            # ONE Exp instruction instead: few loads, early first batch.
            if tt in (1, 3, 7, 11, 15):
                bs = slice({1: 0, 3: 2, 7: 4, 11: 8, 15: 12}[tt], tt + 1)
                nc.scalar.activation(
                    out=lv_all[:, bs], in_=mv_all[:, bs, 1],
                    func=AF.Ln, bias=eps_sb,
                )
                nc.scalar.activation(
                    out=rstd_all[:, bs], in_=lv_all[:, bs],
                    func=AF.Exp, scale=-0.5,
                )
        for tt in range(NT):
            x_tile = x_tiles[tt]
            xf = xfp.tile([P, D], F16, tag="xf")
            nc.vector.tensor_scalar(
                out=xf,
                in0=x_tile,
                scalar1=mv_all[:, tt, 0:1],
                scalar2=rstd_all[:, tt : tt + 1],
                op0=mybir.AluOpType.subtract,
                op1=mybir.AluOpType.mult,
            )
            # 4 transposes into one psum tile, one wide ACT copy out
            psT = psA.tile([P, KT, P], F16, tag="ps")
            for ft in range(KT):
                nc.tensor.transpose(psT[:, ft, :], xf[:, ft * P : (ft + 1) * P], ident)
            nc.scalar.copy(out=xT[:, :, tt * P : (tt + 1) * P], in_=psT)
            # v projection for this token tile (its xT slices just landed)
            if tt >= 2:
                pe_observe(v_aug[:, tt - 2, 0, 0:1])
            ps = psA.tile([P, GCOLS], F32)
            for kt in range(KT):
                nc.tensor.matmul(
                    ps,
                    xT[:, kt, tt * P : (tt + 1) * P],
                    w_sb[:, kt, 2 * GCOLS : 3 * GCOLS],
                    start=(kt == 0),
                    stop=(kt == KT - 1),
                )
            nc.vector.tensor_add(
                out=v_aug[:, tt, :, 0:DH],
                in0=ps.rearrange("p (h d) -> p h d", h=H_PER_CORE),
                in1=vbias_sb.rearrange("p (h d) -> p h d", h=H_PER_CORE),
            )

        # PE observes the final xT copy tick before QKV matmuls
        pe_observe(xT[:, KT - 1, N - 1 : N])

        # ---- Phase B: q/k projections for head-pair 0 ------------------
        def qk_group_steps(mi, nt):
            ps = psA.tile([P, QTW], F32, tag="ps")
            for kt in range(KT):
                yield lambda kt=kt, ps=ps: nc.tensor.matmul(
                    ps,
                    w_sb[:, kt, mi * P : (mi + 1) * P],
                    xT[:, kt, nt * QTW : (nt + 1) * QTW],
                    start=(kt == 0),
                    stop=(kt == KT - 1),
                )
            yield lambda ps=ps: nc.vector.tensor_scalar(
                out=qkT[:, mi, nt * QTW : (nt + 1) * QTW],
                in0=ps,
                scalar1=bias_sb[:, mi : mi + 1],
                scalar2=None,
                op0=mybir.AluOpType.add,
            )

        for mi in (0, 2):
            for step in qk_group_steps(mi, 0):
                step()

        def steps_of(groups):
            for mi, nt in groups:
                yield from qk_group_steps(mi, nt)

        _early = steps_of([(2, 1), (2, 2), (2, 3), (0, 1), (0, 2), (0, 3)])
        _deferred = steps_of([(1, nt) for nt in range(NQT)]
                             + [(3, nt) for nt in range(NQT)])

        pe_observe(qkT[:, 2, QTW - 1 : QTW])
        last_flush = {}
        mid_flush = {}

        # ---- Phase C: attention (S^T layout, head-paired) --------------
        for hp in range(2):
            h0, h1 = 2 * hp, 2 * hp + 1
            mi_q, mi_k = hp, 2 + hp
            for qt in range(NQT):
                qs = slice(qt * QTW, (qt + 1) * QTW)
                po0 = psA.tile([DH + 1, QTW], F32, tag="ps")
                po1 = psA.tile([DH + 1, QTW], F32, tag="ps")
                pending = None
                for kb in range(NT):
                    ks = slice(kb * P, (kb + 1) * P)
                    ps_s = psS.tile([P, 2, QTW], F32)
                    mm_s = nc.tensor.matmul(
                        ps_s[:, 0, :],
                        qkT[0:DH, mi_k, ks],
                        qkT[0:DH, mi_q, qs],
                        start=True,
                        stop=True,
                    )

                    nc.tensor.matmul(
                        ps_s[:, 1, :],
                        qkT[DH:P, mi_k, ks],
                        qkT[DH:P, mi_q, qs],
                        start=True,
                        stop=True,
                    )
                    pT = pP.tile([P, 2, QTW], F16)
                    if kb in DVE_KBS:
                        # Schraudolph: e^(s*SCALE) ~= bitcast_f16(i16(s*A + B))
                        nc.vector.tensor_scalar(
                            out=pT.bitcast(I16),
                            in0=ps_s,
                            scalar1=A_SCHR,
                            scalar2=B_SCHR,
                            op0=mybir.AluOpType.mult,
                            op1=mybir.AluOpType.add,
                        )
                    else:
                        nc.scalar.activation(out=pT, in_=ps_s, func=AF.Exp, scale=SCALE)
                    def pv(pkb, ppT, stop):
                        mm3 = nc.tensor.matmul(
                            po0, v_aug[:, pkb, h0, :], ppT[:, 0, :],
                            start=(pkb == 0), stop=stop,
                        )
                        mm4 = nc.tensor.matmul(
                            po1, v_aug[:, pkb, h1, :], ppT[:, 1, :],
                            start=(pkb == 0), stop=stop,
                        )
                        return mm3, mm4

                    if pending is not None:
                        pkb, ppT = pending
                        if pkb == 0:
                            pe_observe(ppT[:, 0, 0:1])
                        pv(pkb, ppT, stop=False)
                        if hp == 0 and qt == 0:
                            for _ in range(2):
                                step = next(_early, None)
                                if step is not None:
                                    step()
                        elif hp == 0:
                            step = next(_early, None) or next(_deferred, None)
                            if step is not None:
                                step()
                    pending = (kb, pT)
                pkb, ppT = pending
                _, last_att_mm = pv(pkb, ppT, stop=True)
                if hp == 1:
                    last_flush[qt] = last_att_mm
                # normalize both heads (lazy; only gates phase D). Both
                # staging copies go FIRST (frees the PSUM banks promptly);
                # the DMA-gated den chain then runs entirely on GpSimd so
                # the hot ACT/DVE FIFOs never block on a DMA dependency.
                nc.vector.tensor_copy(out=oc_all[:, h0 * NQT + qt, :], in_=po0)
                nc.vector.tensor_copy(out=oc_all[:, h1 * NQT + qt, :], in_=po1)
                for h in (h0, h1):
                    u = h * NQT + qt
                    r0 = (h % 2) * DH
                    if hp == 1 and qt == NQT - 1:
                        # tail fast-path: direct 1-lane reciprocal (no DMA
                        # reshape latency) so phase D's last tiles unblock
                        # sooner after the final attention flush
                        rq = smalls.tile([1, QTW], BF16)
                        with nc.allow_low_precision(reason="softmax denom"):
                            nc.vector.reciprocal(out=rq, in_=oc_all[DH : DH + 1, u, :])
                        rd = dscr.tile([1, QTW], BF16)
                        nc.sync.dma_start(out=rd, in_=rq)
                        rb = smalls.tile([DH, QTW], BF16)
                        nc.sync.dma_start(out=rb, in_=rd.to_broadcast((DH, QTW)))
                        nc.vector.tensor_mul(
                            out=oT[r0 : r0 + DH, h // 2, qs],
                            in0=oc_all[0:DH, u, :],
                            in1=rb,
                        )
                        continue
                    dd = dscr.tile([1, QTW], BF16)
                    nc.sync.dma_start(out=dd, in_=oc_all[DH : DH + 1, u, :])
                    d128 = smalls.tile([P, QTW // P], BF16)
                    nc.sync.dma_start(
                        out=d128, in_=dd.rearrange("o (p j) -> (o p) j", p=P)
                    )
                    r128 = smalls.tile([P, QTW // P], BF16)
                    with nc.allow_low_precision(reason="softmax denom recip"):
                        nc.vector.reciprocal(out=r128, in_=d128)
                    rd = dscr.tile([1, QTW], BF16)
                    nc.sync.dma_start(
                        out=rd.rearrange("o (p j) -> (o p) j", p=P), in_=r128
                    )
                    rb = smalls.tile([DH, QTW], BF16)
                    nc.sync.dma_start(out=rb, in_=rd.to_broadcast((DH, QTW)))
                    nc.vector.tensor_mul(
                        out=oT[r0 : r0 + DH, h // 2, qs],
                        in0=oc_all[0:DH, u, :],
                        in1=rb,
                    )

        # Keep the PE HAM-warm across the normalize-chain tail
        for wk in range(12):
            ps = psA.tile([P, QTW], F32, tag="ps")
            nc.tensor.matmul(
                ps,
                qkT[0:DH, 0, 0:P],
                qkT[0:DH, 0, 0:QTW],
                start=True,
                stop=True,
            )

        pe_observe(oT[0:DH, 1, N - 1 : N].bitcast(F16))

        # ---- Phase D: out projection -----------------------------------
        ob_hist = []
        for tt in range(NT):
            if len(ob_hist) >= 2:
                pe_observe(ob_hist[-2][:, 0:1])
            ps = psA.tile([P, D], F32)
            for ki in range(2):
                mm = nc.tensor.matmul(
                    ps,
                    oT[:, ki, tt * P : (tt + 1) * P],
                    wout_sb[:, ki, :],
                    start=(ki == 0),
                    stop=(ki == 1),
                )
                pin = last_flush[min(tt // NQT + 1, NQT - 1)]
                tile.add_dep_helper(
                    mm.ins, pin.ins, sync=False,
                    reason="phase D after covering attention flush",
                )
            ob = outp.tile([P, D], F32)
            if tt % 2 == 0:
                nc.scalar.copy(out=ob, in_=ps)
            else:
                nc.vector.tensor_copy(out=ob, in_=ps)
            ob_hist.append(ob)
            nc.sync.dma_start(out=out_d[tt * P : (tt + 1) * P, :], in_=ob)

    nc.compile()
    return nc


_NC_CACHE = {}
last_results = None  # BassKernelResults of the most recent run (for test.py)


def _get_nc():
    key = (_DVE_KB_N,)
    if key not in _NC_CACHE:
        _NC_CACHE[key] = _build_nc()
    return _NC_CACHE[key]


def kernel(x, gamma, beta, w_qkv, w_out):
    global last_results
    import ml_dtypes

    x = np.ascontiguousarray(np.asarray(x, dtype=np.float32))
    gamma = np.asarray(gamma, dtype=np.float32)
    beta = np.asarray(beta, dtype=np.float32)
    w_qkv = np.asarray(w_qkv, dtype=np.float32)
    w_out = np.asarray(w_out, dtype=np.float32)

    # fold gamma/beta into the projection (exact algebra)
    wp = gamma[:, None] * w_qkv                      # [512, 1536]
    bp = beta @ w_qkv                                # [1536]

    in_maps = []
    for c in range(N_CORES):
        b = c // 2
        g = c % 2
        sl = [slice(s * D + g * GCOLS, s * D + (g + 1) * GCOLS) for s in range(3)]
        wg = np.concatenate([wp[:, s] for s in sl], axis=1)          # [512, 768]
        bg = np.concatenate([bp[s] for s in sl])[:, None]            # [768, 1]
        wo = w_out[g * GCOLS : (g + 1) * GCOLS, :]                   # [256, 512]
        in_maps.append(
            {
                "x": np.ascontiguousarray(x[b]),
                "wqkv": np.ascontiguousarray(wg.astype(np.float16)),
                "bqkv": np.ascontiguousarray(bg.astype(np.float32)),
                "wout": np.ascontiguousarray(wo.astype(ml_dtypes.bfloat16)),
            }
        )

    nc = _get_nc()
    last_results = run_bass_kernel_spmd(nc, in_maps, list(range(N_CORES)))
    outs = [m["out"] for m in last_results.results]
    out = np.stack([outs[2 * b] + outs[2 * b + 1] for b in range(B)])
    return np.ascontiguousarray(out.astype(np.float32))
